# revision 9
# baseline (speedup 1.0000x reference)
"""Trainium2 Bass kernel for nn_CCN1D (circulant GNN message passing).

Strategy
--------
The reference gathers receptive fields on a circulant ring graph and runs
per-edge MLPs followed by segment sums.  Because every gathered row's MLP
output depends only on the *source* vertex, the per-edge MLPs (130k / 250k
rows) collapse to per-vertex MLPs (10k rows) plus sliding-window sums along
the ring:

    dense = relu(X @ W1 + b1)                           [N, 128]
    z_f[u]  = relu(relu(dense[u] @ (w0a_lo+w0a_hi)/13) @ w0b)      [N, 64]
    s0_f[v] = sum_{j=0..12} z_f[(v+j) % N]              (window sum)
    z1_f[u] = relu(relu(concat(s0_f[u], z_f[u])/25 @ w1a) @ w1b)
    s1_f[v] = sum_{j=0..24} z1_f[(v+j) % N]
    (reverse branch identical with backward windows)
    logits  = concat(dense, s0f, s1f, s0r, s1r) @ W2 + b2
    out     = log_softmax(logits) * mask

Sharding: vertices are range-partitioned across 8 cores with a 36-vertex
halo on each side (graph/data parallel; weights replicated; no device
collectives needed - the halo makes every core self-sufficient).

v2 implementation notes (vs the earlier f32r version):
- all matmul operands are bf16 (X, weights, activations): halves all HBM
  traffic; window prefix sums stay f32.  rel-err ~1.5e-3, well inside 2e-2.
- layer-1 contraction split into two 64-row matmuls (s0-part from the S0
  tile, z-part from the Z tile) so the s0f/s0r window outputs live in one
  128-partition tile; fc2 then needs only 3 passes (D, S0, S1).
- branch f/r PSUM outputs paired in one two-bank PSUM tile so a single
  activation op evicts both.
- log_softmax skips the max-subtraction (fp32 exp is safe at this logit
  scale) and runs per col-tile on the transposed PSUM banks.
- output written per-partition-contiguous ([128, 11, 16]); the host
  de-interleaves lanes back to row-major.
- edge lanes handled by tiny memsets; out-of-range lanes are discarded by
  the host gather.
"""

import sys

import numpy as np

for _p in ("/opt/trn_rl_repo",):
    if _p not in sys.path:
        sys.path.insert(0, _p)

import ml_dtypes

N = 10000
NCORES = 8
BLK = N // NCORES          # 1250 vertices per core
HALO = 36                  # 12 (layer-0 window) + 24 (layer-1 window)
W = 1344                   # on-chip free width (1322 valid + pad)
NT = 11                    # 128-lane row tiles covering W (10*128 + 64)
CTS = ((0, 512), (512, 512), (1024, 320))
RF1, RF2 = 13, 25
C_IN, C_HID, MLP_H, MSG, NCLS = 512, 128, 128, 64, 16
LO, HI = HALO, HALO + BLK  # valid output lane range [36, 1286)
WPACK_COLS = 1328          # packed bf16 matmul weights
CPACK_COLS = 2 + NT + NCLS  # biases + mask + identity (f32)

_F32 = np.float32
_BF16 = ml_dtypes.bfloat16


# --------------------------------------------------------------------------
# structure check (is the input the circulant graph the kernel was built for?)
# --------------------------------------------------------------------------

def _expected_idx():
    v = np.arange(N)
    return {
        "f_rf1": ((v[:, None] + np.arange(RF1)) % N).reshape(-1),
        "f_rf2": ((v[:, None] + np.arange(RF2)) % N).reshape(-1),
        "r_rf1": ((v[:, None] - np.arange(RF1)) % N).reshape(-1),
        "r_rf2": ((v[:, None] - np.arange(RF2)) % N).reshape(-1),
        "own1": np.repeat(v, RF1),
        "own2": np.repeat(v, RF2),
        "self1": v * RF1,
    }


def _structure_matches(inputs):
    try:
        if inputs["sparse_feature"].shape != (N, C_IN):
            return False
        for k, exp in _expected_idx().items():
            got = np.asarray(inputs[k])
            if got.shape != exp.shape or not np.array_equal(got, exp):
                return False
        return True
    except Exception:
        return False


# --------------------------------------------------------------------------
# generic numpy fallback (exact reference semantics, any index content)
# --------------------------------------------------------------------------

def _segment_sum(data, seg, num):
    out = np.zeros((num,) + data.shape[1:], dtype=data.dtype)
    np.add.at(out, seg, data)
    return out


def _np_branch(dense, rf1, rf2, own1, own2, self1, w0a, w0b, w1a, w1b):
    sizes1 = _segment_sum(np.ones(own1.shape, dense.dtype), own1, N)
    sizes2 = _segment_sum(np.ones(own2.shape, dense.dtype), own2, N)
    g = dense[rf1]
    m0 = np.concatenate([g, g], axis=-1) / sizes1[own1][:, None]
    h0 = np.maximum(np.maximum(m0 @ w0a, 0.0) @ w0b, 0.0)
    s0 = _segment_sum(h0, own1, N)
    selfr = h0[self1]
    m1 = np.concatenate([s0[rf2], selfr[rf2]], axis=-1) / sizes2[own2][:, None]
    h1 = np.maximum(np.maximum(m1 @ w1a, 0.0) @ w1b, 0.0)
    s1 = _segment_sum(h1, own2, N)
    return s0, s1


def _reference_numpy(inputs):
    f = {k: np.asarray(v) for k, v in inputs.items()}
    dense = np.maximum(
        f["sparse_feature"].astype(_F32) @ f["fc1_w"] + f["fc1_b"], 0.0
    )
    s0f, s1f = _np_branch(dense, f["f_rf1"], f["f_rf2"], f["own1"], f["own2"],
                          f["self1"], f["mw0a"], f["mw0b"], f["mw1a"], f["mw1b"])
    s0r, s1r = _np_branch(dense, f["r_rf1"], f["r_rf2"], f["own1"], f["own2"],
                          f["self1"], f["rw0a"], f["rw0b"], f["rw1a"], f["rw1b"])
    total = np.concatenate([dense, s0f, s1f, s0r, s1r], axis=1)
    logits = total @ f["fc2_w"] + f["fc2_b"]
    m = logits.max(axis=-1, keepdims=True)
    lse = m + np.log(np.exp(logits - m).sum(axis=-1, keepdims=True))
    return ((logits - lse) * f["mask"][:, None].astype(_F32)).astype(_F32)


# --------------------------------------------------------------------------
# device kernel
# --------------------------------------------------------------------------

_NC = None


def _build_nc(repeat=1):
    import concourse.bass as bass
    import concourse.tile as tile
    from concourse import bacc, mybir

    f32 = mybir.dt.float32
    bf16 = mybir.dt.bfloat16
    AF = mybir.ActivationFunctionType
    OP = mybir.AluOpType

    nc = bacc.Bacc(trn_type="TRN2", debug=False)

    xt_d = nc.dram_tensor("xt", [C_IN, W], bf16, kind="ExternalInput").ap()
    wpack_d = nc.dram_tensor("wpack", [128, WPACK_COLS], bf16,
                             kind="ExternalInput").ap()
    cpack_d = nc.dram_tensor("cpack", [128, CPACK_COLS], f32,
                             kind="ExternalInput").ap()
    out_d = nc.dram_tensor("out", [128, NT, NCLS], f32,
                           kind="ExternalOutput").ap()

    with tile.TileContext(nc) as tc:
        from contextlib import ExitStack

        with ExitStack() as ctx:
            cp = ctx.enter_context(tc.tile_pool(name="consts", bufs=1))
            ap_ = ctx.enter_context(tc.tile_pool(name="acts", bufs=1))
            sp = ctx.enter_context(tc.tile_pool(name="scr", bufs=3))
            pmm = ctx.enter_context(tc.tile_pool(name="pmm", bufs=2, space="PSUM"))
            pzz = ctx.enter_context(tc.tile_pool(name="pzz", bufs=1, space="PSUM"))
            pl = ctx.enter_context(tc.tile_pool(name="pl", bufs=1, space="PSUM"))
            pt = ctx.enter_context(tc.tile_pool(name="pt", bufs=2, space="PSUM"))

            for _rep in range(repeat):
              # ---- const DMAs first (tiny cpack feeds the PE warm-up) ----
              cpack = cp.tile([128, CPACK_COLS], f32, tag="cpack", name="cpack")
              nc.sync.dma_start(out=cpack, in_=cpack_d)
              wpack = cp.tile([128, WPACK_COLS], bf16, tag="wpack", name="wpack")
              nc.sync.dma_start(out=wpack, in_=wpack_d)

              bfc1 = cpack[:, 0:1]
              bfc2 = cpack[0:16, 1:2]
              maskv = cpack[:, 2:2 + NT]
              ident = cpack[0:16, 2 + NT:2 + NT + NCLS]

              # PE warm-up on cpack garbage (f32 = 4 cycles/row keeps the PE
              # busy through its p-state ramp while X streams in).  ifmap is
              # a stride-0 broadcast of one cpack column.
              def bcast_free(t2d, m):
                  return bass.AP(tensor=t2d.tensor, offset=t2d.offset,
                                 ap=[t2d.ap[0], [0, m]])

              warm = pl.tile([NCLS, 512], f32, tag="psL", name="warm")
              nc.tensor.matmul(warm[:, 0:128], cpack[:, 13:29],
                               bcast_free(cpack[:, 0:1], 128),
                               start=True, stop=True, skip_group_check=True)
              nc.tensor.matmul(warm[:, 128:256], cpack[:, 13:29],
                               bcast_free(cpack[:, 0:1], 128),
                               start=True, stop=True, skip_group_check=True)

              # ---- X: one DMA per col-tile (all four K-chunks) ----
              xt_pack = cp.tile([128, 4, W], bf16, tag="xtp", name="xt_pack")
              xt = [xt_pack[:, k, :] for k in range(4)]
              xt_k = xt_d.rearrange("(k p) w -> p k w", p=128)
              for s, w in CTS:
                  nc.sync.dma_start(out=xt_pack[:, :, s:s + w],
                                    in_=xt_k[:, :, s:s + w])

              # ---- weight views (all bf16) ----
              wfc1 = [wpack[:, 128 * k:128 * (k + 1)] for k in range(4)]
              wz = {"f": wpack[:, 512:640], "r": wpack[:, 640:768]}
              wzb = {"f": wpack[:, 768:832], "r": wpack[:, 832:896]}
              wz1hi = {"f": wpack[0:64, 896:1024], "r": wpack[64:128, 896:1024]}
              wz1lo = {"f": wpack[0:64, 1024:1152], "r": wpack[64:128, 1024:1152]}
              wz1b = {"f": wpack[:, 1152:1216], "r": wpack[:, 1216:1280]}
              w2d = wpack[:, 1280:1296]
              w2s0 = wpack[:, 1296:1312]
              w2s1 = wpack[:, 1312:1328]

              # ---- persistent activation tiles ----
              D = ap_.tile([128, W], bf16, tag="D")
              Z = ap_.tile([128, W], bf16, tag="Z")    # [0:64]=z_f, [64:128]=z_r
              ZZ = ap_.tile([128, W], bf16, tag="ZZ")  # z1_f, z1_r
              P13 = ap_.tile([128, W], f32, tag="P13")
              P25 = ap_.tile([128, W], f32, tag="P25")
              S0 = ap_.tile([128, W], bf16, tag="S0")  # [0:64]=s0f, [64:128]=s0r
              S1 = ap_.tile([128, W], bf16, tag="S1")  # [0:64]=s1f, [64:128]=s1r
              Lsb = ap_.tile([NCLS, W], f32, tag="Lsb")
              LT = ap_.tile([128, NT, NCLS], f32, tag="LT")
              se = ap_.tile([128, NT], f32, tag="se")

              # edge lanes the window subs can't reach (values unused; they
              # only need to be finite so the chained scans stay clean)
              nc.gpsimd.memset(S0[0:64, 0:1], 0.0)
              nc.gpsimd.memset(S0[64:128, 0:13], 0.0)
              nc.gpsimd.memset(S1[0:64, 0:1], 0.0)
              nc.gpsimd.memset(S1[64:128, 0:25], 0.0)

              # ---- stage A+B per col-tile: fc1, layer-0 MLP, chained scan --
              def stage_ab(j):
                  s, w = CTS[j]
                  psA = pmm.tile([128, 2, 512], f32, tag="mm", name="psA")
                  for k in range(4):
                      nc.tensor.matmul(psA[:, 0, :w], wfc1[k], xt[k][:, s:s + w],
                                       start=(k == 0), stop=(k == 3))
                  nc.scalar.activation(D[:, s:s + w], psA[:, 0, :w], AF.Relu,
                                       bias=bfc1)
                  t1p = pmm.tile([128, 2, 512], f32, tag="mm", name="t1p")
                  nc.tensor.matmul(t1p[:, 0, :w], wz["f"], D[:, s:s + w],
                                   start=True, stop=True)
                  nc.tensor.matmul(t1p[:, 1, :w], wz["r"], D[:, s:s + w],
                                   start=True, stop=True)
                  t1s = sp.tile([128, 2, 512], bf16, tag="t1s", name="t1s")
                  nc.scalar.activation(t1s[:, :, :w], t1p[:, :, :w], AF.Relu)
                  zp = pzz.tile([128, 512], f32, tag="zz", name="zp")
                  nc.tensor.matmul(zp[0:64, :w], wzb["f"], t1s[:, 0, :w],
                                   start=True, stop=True)
                  nc.tensor.matmul(zp[64:128, :w], wzb["r"], t1s[:, 1, :w],
                                   start=True, stop=True)
                  nc.vector.tensor_scalar_max(Z[:, s:s + w], zp[:, :w], 0.0)
                  nc.vector.tensor_tensor_scan(
                      P13[:, s:s + w], Z[:, s:s + w], Z[:, s:s + w],
                      initial=(0.0 if s == 0 else P13[:, s - 1:s]),
                      op0=OP.add, op1=OP.bypass)

              # ---- stage C: s0 windows from shifted prefix differences ----
              def win13(j):
                  s, w = CTS[j]
                  a, b = (1 if s == 0 else s), min(s + w, 1332)
                  nc.vector.tensor_sub(S0[0:64, a:b], P13[0:64, a + 12:b + 12],
                                       P13[0:64, a - 1:b - 1])
                  a, b = (13 if s == 0 else s), s + w
                  nc.gpsimd.tensor_sub(S0[64:128, a:b], P13[64:128, a:b],
                                       P13[64:128, a - 13:b - 13])

              # ---- stage D: layer-1 MLP (split contraction) + chained scan -
              def stage_d(j):
                  s, w = CTS[j]
                  t2p = pmm.tile([128, 2, 512], f32, tag="mm", name="t2p")
                  for i, br in enumerate("fr"):
                      h = slice(0, 64) if br == "f" else slice(64, 128)
                      nc.tensor.matmul(t2p[:, i, :w], wz1hi[br], S0[h, s:s + w],
                                       start=True, stop=False)
                      nc.tensor.matmul(t2p[:, i, :w], wz1lo[br], Z[h, s:s + w],
                                       start=False, stop=True)
                  t2s = sp.tile([128, 2, 512], bf16, tag="t1s", name="t2s")
                  nc.scalar.activation(t2s[:, :, :w], t2p[:, :, :w], AF.Relu)
                  z1p = pzz.tile([128, 512], f32, tag="zz", name="z1p")
                  nc.tensor.matmul(z1p[0:64, :w], wz1b["f"], t2s[:, 0, :w],
                                   start=True, stop=True)
                  nc.tensor.matmul(z1p[64:128, :w], wz1b["r"], t2s[:, 1, :w],
                                   start=True, stop=True)
                  nc.vector.tensor_scalar_max(ZZ[:, s:s + w], z1p[:, :w], 0.0)
                  nc.vector.tensor_tensor_scan(
                      P25[:, s:s + w], ZZ[:, s:s + w], ZZ[:, s:s + w],
                      initial=(0.0 if s == 0 else P25[:, s - 1:s]),
                      op0=OP.add, op1=OP.bypass)

              # ---- stage E: s1 windows ----
              def win25(j):
                  s, w = CTS[j]
                  a, b = (1 if s == 0 else s), min(s + w, 1320)
                  nc.vector.tensor_sub(S1[0:64, a:b], P25[0:64, a + 24:b + 24],
                                       P25[0:64, a - 1:b - 1])
                  a, b = (25 if s == 0 else s), s + w
                  nc.gpsimd.tensor_sub(S1[64:128, a:b], P25[64:128, a:b],
                                       P25[64:128, a - 25:b - 25])

              # ---- stage F: fc2 + logits eviction ----
              BLKS = ((0, 4), (4, 4), (8, 3))  # (first 128-block, n blocks)

              def bcast(t2d, n):
                  return bass.AP(tensor=t2d.tensor, offset=t2d.offset,
                                 ap=[t2d.ap[0], [t2d.ap[1][0], n], [0, NCLS]])

              psTs = {}

              def stage_f(j):
                  s, w = CTS[j]
                  psl = pl.tile([NCLS, 512], f32, tag="psL", name="psl")
                  for i, (wc, rhs) in enumerate((
                          (w2d, D[:, s:s + w]),
                          (w2s0, S0[:, s:s + w]),
                          (w2s1, S1[:, s:s + w]))):
                      nc.tensor.matmul(psl[:, :w], wc, rhs,
                                       start=(i == 0), stop=(i == 2))
                  nc.scalar.activation(Lsb[:, s:s + w], psl[:, :w], AF.Identity,
                                       bias=bfc2)

              # ---- stage G: transpose, log_softmax (no max-sub), out DMA ----
              def softmax_chunk(psT, t0, c0, nb):
                  # psT holds blocks [t0, t0+...); this chunk covers
                  # blocks [t0+c0, t0+c0+nb)
                  ex = sp.tile([128, 4, NCLS], f32, tag="ex", name="ex")
                  seh = se[:, t0 + c0:t0 + c0 + nb]
                  nc.scalar.activation(ex[:, 0:nb, :], psT[:, c0:c0 + nb, :],
                                       AF.Exp)
                  nc.vector.reduce_sum(seh, ex[:, 0:nb, :],
                                       axis=mybir.AxisListType.X)
                  nc.scalar.activation(seh, seh, AF.Ln)
                  lt = LT[:, t0 + c0:t0 + c0 + nb, :]
                  nc.vector.tensor_sub(lt, psT[:, c0:c0 + nb, :], bcast(seh, nb))
                  nc.gpsimd.tensor_mul(
                      lt, lt, bcast(maskv[:, t0 + c0:t0 + c0 + nb], nb))
                  nc.sync.dma_start(out=out_d[:, t0 + c0:t0 + c0 + nb, :],
                                    in_=lt)

              def stage_g_transpose(j):
                  t0, nb = BLKS[j]
                  psT = pt.tile([128, 4, NCLS], f32, tag="psT", name="psT")
                  psTs[j] = psT
                  if j == 2:
                      nc.vector.memset(psT[64:128, nb - 1, :], 0.0)
                  for t in range(t0, t0 + nb):
                      wt = 128 if t < NT - 1 else W - 128 * (NT - 1)
                      nc.tensor.transpose(psT[:wt, t - t0, :],
                                          Lsb[:, 128 * t:128 * t + wt], ident)

              def stage_g_softmax(j):
                  t0, nb = BLKS[j]
                  if j < 2:
                      softmax_chunk(psTs[j], t0, 0, nb)
                  else:
                      softmax_chunk(psTs[j], t0, 0, nb - 1)
                      softmax_chunk(psTs[j], t0, nb - 1, 1)

              # ---- emission order: dense PE stream, one-tile-lag windows ----
              stage_ab(0)
              stage_ab(1)
              win13(0)
              stage_ab(2)
              win13(1)
              stage_d(0)
              stage_d(1)
              win25(0)
              win13(2)
              stage_d(2)
              stage_f(0)
              win25(1)
              stage_f(1)
              stage_g_transpose(0)
              stage_g_softmax(0)
              win25(2)
              stage_f(2)
              stage_g_transpose(1)
              stage_g_softmax(1)
              stage_g_transpose(2)
              stage_g_softmax(2)

    # Steer the ACT-table pass to natural_log_exp_and_others (covers Relu,
    # Identity, Copy, Exp AND Ln) so the kernel pays one table load instead
    # of a ~2.7us mid-kernel switch before the final Ln.
    import concourse.bacc as bacc_mod
    AFt = mybir.ActivationFunctionType

    orig_tables = bacc_mod.get_activation_tables
    mine = {AFt.Relu, AFt.Identity, AFt.Copy, AFt.Exp, AFt.Ln}

    def steered(arch):
        t = orig_tables(arch)
        out = {}
        seen_pref = False
        for name, fns in t.items():
            if name == "natural_log_exp_and_others":
                seen_pref = True
                out[name] = fns
            elif not seen_pref:
                out[name] = type(fns)(f for f in fns if f not in mine)
            else:
                out[name] = fns
        return out

    bacc_mod.get_activation_tables = steered
    try:
        nc.compile()
    finally:
        bacc_mod.get_activation_tables = orig_tables
    return nc


def _get_nc(repeat=1):
    global _NC
    if repeat != 1:
        return _build_nc(repeat)
    if _NC is None:
        _NC = _build_nc()
    return _NC


# --------------------------------------------------------------------------
# host-side sharding + entry point
# --------------------------------------------------------------------------

def _make_in_maps(inputs):
    sf = np.asarray(inputs["sparse_feature"], dtype=_F32)
    maskf = np.asarray(inputs["mask"]).astype(_F32)

    def f(k):
        return np.asarray(inputs[k], dtype=_F32)

    mw0a, rw0a = f("mw0a"), f("rw0a")
    wpack = np.zeros((128, WPACK_COLS), dtype=_F32)
    wpack[:, 0:512] = f("fc1_w").reshape(4, 128, C_HID).transpose(1, 0, 2) \
        .reshape(128, 512)
    wpack[:, 512:640] = (mw0a[:C_HID] + mw0a[C_HID:]) / RF1
    wpack[:, 640:768] = (rw0a[:C_HID] + rw0a[C_HID:]) / RF1
    wpack[:, 768:832] = f("mw0b")
    wpack[:, 832:896] = f("rw0b")
    mw1a, rw1a = f("mw1a") / RF2, f("rw1a") / RF2
    wpack[0:64, 896:1024] = mw1a[0:64]      # s0-part, branch f (parts 0:64)
    wpack[64:128, 896:1024] = rw1a[0:64]    # s0-part, branch r (parts 64:128)
    wpack[0:64, 1024:1152] = mw1a[64:128]   # z-part, branch f
    wpack[64:128, 1024:1152] = rw1a[64:128]  # z-part, branch r
    wpack[:, 1152:1216] = f("mw1b")
    wpack[:, 1216:1280] = f("rw1b")
    w2 = f("fc2_w")
    wpack[:, 1280:1296] = w2[0:128]
    wpack[0:64, 1296:1312] = w2[128:192]    # s0f rows
    wpack[64:128, 1296:1312] = w2[256:320]  # s0r rows
    wpack[0:64, 1312:1328] = w2[192:256]    # s1f rows
    wpack[64:128, 1312:1328] = w2[320:384]  # s1r rows
    wpack16 = wpack.astype(_BF16)

    cbase = np.zeros((128, CPACK_COLS), dtype=_F32)
    cbase[:, 0] = f("fc1_b")
    cbase[0:NCLS, 1] = f("fc2_b")
    cbase[0:NCLS, 2 + NT:2 + NT + NCLS] = np.eye(NCLS, dtype=_F32)

    in_maps = []
    for c in range(NCORES):
        b = c * BLK
        idx = (b - HALO + np.arange(W)) % N
        xt = np.ascontiguousarray(sf[idx].T.astype(_BF16))
        me = np.zeros(128 * NT, dtype=_F32)
        me[:W] = maskf[idx]
        cpack = cbase.copy()
        cpack[:, 2:2 + NT] = me.reshape(NT, 128).T
        in_maps.append({"wpack": wpack16, "cpack": cpack, "xt": xt})
    return in_maps


_RUNNER = None


def _make_runner():
    """Build the 8-core PJRT executor once; reuse across kernel() calls."""
    import jax
    from jax.sharding import Mesh, NamedSharding, PartitionSpec
    from jax.experimental.shard_map import shard_map
    from concourse import mybir
    from concourse.bass2jax import (_bass_exec_p, install_neuronx_cc_hook,
                                    partition_id_tensor)

    nc = _get_nc()
    install_neuronx_cc_hook()
    in_names, out_names, out_avals, zero_shapes = [], [], [], []
    pname = nc.partition_id_tensor.name if nc.partition_id_tensor else None
    for alloc in nc.m.functions[0].allocations:
        if not isinstance(alloc, mybir.MemoryLocationSet):
            continue
        name = alloc.memorylocations[0].name
        if alloc.kind == "ExternalInput":
            if name != pname:
                in_names.append(name)
        elif alloc.kind == "ExternalOutput":
            out_names.append(name)
            shape = tuple(alloc.tensor_shape)
            dtype = mybir.dt.np(alloc.dtype)
            out_avals.append(jax.core.ShapedArray(shape, dtype))
            zero_shapes.append((shape, dtype))
    n_params = len(in_names)
    all_in = list(in_names) + list(out_names)
    if pname is not None:
        all_in.append(pname)
    donate = tuple(range(n_params, n_params + len(out_names)))

    def _body(*args):
        operands = list(args)
        if pname is not None:
            operands.append(partition_id_tensor())
        return tuple(_bass_exec_p.bind(
            *operands,
            out_avals=tuple(out_avals),
            in_names=tuple(all_in),
            out_names=tuple(out_names),
            lowering_input_output_aliases=(),
            sim_require_finite=False,
            sim_require_nnan=False,
            nc=nc,
        ))

    devices = jax.devices()[:NCORES]
    mesh = Mesh(np.asarray(devices), ("core",))
    shd = NamedSharding(mesh, PartitionSpec("core"))
    n_outs = len(out_names)
    sharded = jax.jit(
        shard_map(_body, mesh=mesh,
                  in_specs=(PartitionSpec("core"),) * (n_params + n_outs),
                  out_specs=(PartitionSpec("core"),) * n_outs,
                  check_rep=False),
        donate_argnums=donate, keep_unused=True,
    )

    def run(in_maps):
        concat_in = [
            np.concatenate([np.asarray(in_maps[c][nm]) for c in range(NCORES)],
                           axis=0)
            for nm in in_names
        ]
        dev_in = [jax.device_put(x, shd) for x in concat_in]
        zeros = [
            jax.device_put(np.zeros((NCORES * s[0], *s[1:]), dt), shd)
            for s, dt in zero_shapes
        ]
        outs = sharded(*dev_in, *zeros)
        res = np.asarray(outs[out_names.index("out")])
        # res: [NCORES*128, NT, 16] -> per core [128, NT, 16]; lane = t*128+p
        res = res.reshape(NCORES, 128, NT, NCLS).transpose(0, 2, 1, 3) \
                 .reshape(NCORES, NT * 128, NCLS)
        return np.ascontiguousarray(res[:, LO:HI, :]).reshape(N, NCLS)

    return run


def kernel(**inputs):
    if not _structure_matches(inputs):
        return _reference_numpy(inputs)
    global _RUNNER
    if _RUNNER is None:
        _RUNNER = _make_runner()
    return _RUNNER(_make_in_maps(inputs))


# revision 11
# speedup vs baseline: 3.8754x; 3.8754x over previous
"""Trainium2 Bass kernel for nn_CCN1D (circulant GNN message passing).

Strategy
--------
The reference gathers receptive fields on a circulant ring graph and runs
per-edge MLPs followed by segment sums.  Because every gathered row's MLP
output depends only on the *source* vertex, the per-edge MLPs (130k / 250k
rows) collapse to per-vertex MLPs (10k rows) plus sliding-window sums along
the ring:

    dense = relu(X @ W1 + b1)                           [N, 128]
    z_f[u]  = relu(relu(dense[u] @ (w0a_lo+w0a_hi)/13) @ w0b)      [N, 64]
    s0_f[v] = sum_{j=0..12} z_f[(v+j) % N]              (window sum)
    z1_f[u] = relu(relu(concat(s0_f[u], z_f[u])/25 @ w1a) @ w1b)
    s1_f[v] = sum_{j=0..24} z1_f[(v+j) % N]
    (reverse branch identical with backward windows)
    logits  = concat(dense, s0f, s1f, s0r, s1r) @ W2 + b2
    out     = log_softmax(logits) * mask

Sharding: vertices are range-partitioned across 8 cores with a 36-vertex
halo on each side (graph/data parallel; weights replicated; no device
collectives needed - the halo makes every core self-sufficient).

v2 implementation notes (vs the earlier f32r version):
- all matmul operands are bf16 (X, weights, activations): halves all HBM
  traffic; window prefix sums stay f32.  rel-err ~1.5e-3, well inside 2e-2.
- layer-1 contraction split into two 64-row matmuls (s0-part from the S0
  tile, z-part from the Z tile) so the s0f/s0r window outputs live in one
  128-partition tile; fc2 then needs only 3 passes (D, S0, S1).
- branch f/r PSUM outputs paired in one two-bank PSUM tile so a single
  activation op evicts both.
- log_softmax skips the max-subtraction (fp32 exp is safe at this logit
  scale) and runs per col-tile on the transposed PSUM banks.
- output written per-partition-contiguous ([128, 11, 16]); the host
  de-interleaves lanes back to row-major.
- edge lanes handled by tiny memsets; out-of-range lanes are discarded by
  the host gather.
"""

import sys

import numpy as np

for _p in ("/opt/trn_rl_repo",):
    if _p not in sys.path:
        sys.path.insert(0, _p)

import ml_dtypes

N = 10000
NCORES = 8
BLK = N // NCORES          # 1250 vertices per core
HALO = 36                  # 12 (layer-0 window) + 24 (layer-1 window)
W = 1344                   # on-chip free width (1322 valid + pad)
NT = 11                    # 128-lane row tiles covering W (10*128 + 64)
CTS = ((0, 512), (512, 512), (1024, 320))
RF1, RF2 = 13, 25
C_IN, C_HID, MLP_H, MSG, NCLS = 512, 128, 128, 64, 16
LO, HI = HALO, HALO + BLK  # valid output lane range [36, 1286)
WPACK_COLS = 1328          # packed bf16 matmul weights
CPACK_COLS = 2 + NT + NCLS  # biases + mask + identity (f32)

_F32 = np.float32
_BF16 = ml_dtypes.bfloat16


# --------------------------------------------------------------------------
# structure check (is the input the circulant graph the kernel was built for?)
# --------------------------------------------------------------------------

def _expected_idx():
    v = np.arange(N)
    return {
        "f_rf1": ((v[:, None] + np.arange(RF1)) % N).reshape(-1),
        "f_rf2": ((v[:, None] + np.arange(RF2)) % N).reshape(-1),
        "r_rf1": ((v[:, None] - np.arange(RF1)) % N).reshape(-1),
        "r_rf2": ((v[:, None] - np.arange(RF2)) % N).reshape(-1),
        "own1": np.repeat(v, RF1),
        "own2": np.repeat(v, RF2),
        "self1": v * RF1,
    }


def _structure_matches(inputs):
    try:
        if inputs["sparse_feature"].shape != (N, C_IN):
            return False
        for k, exp in _expected_idx().items():
            got = np.asarray(inputs[k])
            if got.shape != exp.shape or not np.array_equal(got, exp):
                return False
        return True
    except Exception:
        return False


# --------------------------------------------------------------------------
# generic numpy fallback (exact reference semantics, any index content)
# --------------------------------------------------------------------------

def _segment_sum(data, seg, num):
    out = np.zeros((num,) + data.shape[1:], dtype=data.dtype)
    np.add.at(out, seg, data)
    return out


def _np_branch(dense, rf1, rf2, own1, own2, self1, w0a, w0b, w1a, w1b):
    sizes1 = _segment_sum(np.ones(own1.shape, dense.dtype), own1, N)
    sizes2 = _segment_sum(np.ones(own2.shape, dense.dtype), own2, N)
    g = dense[rf1]
    m0 = np.concatenate([g, g], axis=-1) / sizes1[own1][:, None]
    h0 = np.maximum(np.maximum(m0 @ w0a, 0.0) @ w0b, 0.0)
    s0 = _segment_sum(h0, own1, N)
    selfr = h0[self1]
    m1 = np.concatenate([s0[rf2], selfr[rf2]], axis=-1) / sizes2[own2][:, None]
    h1 = np.maximum(np.maximum(m1 @ w1a, 0.0) @ w1b, 0.0)
    s1 = _segment_sum(h1, own2, N)
    return s0, s1


def _reference_numpy(inputs):
    f = {k: np.asarray(v) for k, v in inputs.items()}
    dense = np.maximum(
        f["sparse_feature"].astype(_F32) @ f["fc1_w"] + f["fc1_b"], 0.0
    )
    s0f, s1f = _np_branch(dense, f["f_rf1"], f["f_rf2"], f["own1"], f["own2"],
                          f["self1"], f["mw0a"], f["mw0b"], f["mw1a"], f["mw1b"])
    s0r, s1r = _np_branch(dense, f["r_rf1"], f["r_rf2"], f["own1"], f["own2"],
                          f["self1"], f["rw0a"], f["rw0b"], f["rw1a"], f["rw1b"])
    total = np.concatenate([dense, s0f, s1f, s0r, s1r], axis=1)
    logits = total @ f["fc2_w"] + f["fc2_b"]
    m = logits.max(axis=-1, keepdims=True)
    lse = m + np.log(np.exp(logits - m).sum(axis=-1, keepdims=True))
    return ((logits - lse) * f["mask"][:, None].astype(_F32)).astype(_F32)


# --------------------------------------------------------------------------
# device kernel
# --------------------------------------------------------------------------

_NC = None


def _build_nc(repeat=1):
    import concourse.bass as bass
    import concourse.tile as tile
    from concourse import bacc, mybir

    f32 = mybir.dt.float32
    bf16 = mybir.dt.bfloat16
    AF = mybir.ActivationFunctionType
    OP = mybir.AluOpType

    nc = bacc.Bacc(trn_type="TRN2", debug=False)

    xt_d = nc.dram_tensor("xt", [C_IN, W], bf16, kind="ExternalInput").ap()
    wpack_d = nc.dram_tensor("wpack", [128, WPACK_COLS], bf16,
                             kind="ExternalInput").ap()
    cpack_d = nc.dram_tensor("cpack", [128, CPACK_COLS], f32,
                             kind="ExternalInput").ap()
    out_d = nc.dram_tensor("out", [128, NT, NCLS], f32,
                           kind="ExternalOutput").ap()

    with tile.TileContext(nc) as tc:
        from contextlib import ExitStack

        with ExitStack() as ctx:
            cp = ctx.enter_context(tc.tile_pool(name="consts", bufs=1))
            ap_ = ctx.enter_context(tc.tile_pool(name="acts", bufs=1))
            sp = ctx.enter_context(tc.tile_pool(name="scr", bufs=3))
            pmm = ctx.enter_context(tc.tile_pool(name="pmm", bufs=2, space="PSUM"))
            pzz = ctx.enter_context(tc.tile_pool(name="pzz", bufs=1, space="PSUM"))
            pl = ctx.enter_context(tc.tile_pool(name="pl", bufs=1, space="PSUM"))
            pt = ctx.enter_context(tc.tile_pool(name="pt", bufs=2, space="PSUM"))

            for _rep in range(repeat):
              # ---- const DMAs first (tiny cpack feeds the PE warm-up) ----
              cpack = cp.tile([128, CPACK_COLS], f32, tag="cpack", name="cpack")
              nc.sync.dma_start(out=cpack, in_=cpack_d)
              wpack = cp.tile([128, WPACK_COLS], bf16, tag="wpack", name="wpack")
              nc.sync.dma_start(out=wpack, in_=wpack_d)

              bfc1 = cpack[:, 0:1]
              bfc2 = cpack[0:16, 1:2]
              maskv = cpack[:, 2:2 + NT]
              ident = cpack[0:16, 2 + NT:2 + NT + NCLS]

              # PE warm-up on cpack garbage (f32 = 4 cycles/row keeps the PE
              # busy through its p-state ramp while X streams in).  ifmap is
              # a stride-0 broadcast of one cpack column.
              def bcast_free(t2d, m):
                  return bass.AP(tensor=t2d.tensor, offset=t2d.offset,
                                 ap=[t2d.ap[0], [0, m]])

              warm = pl.tile([NCLS, 512], f32, tag="psL", name="warm")
              nc.tensor.matmul(warm[:, 0:128], cpack[:, 13:29],
                               bcast_free(cpack[:, 0:1], 128),
                               start=True, stop=True, skip_group_check=True)
              nc.tensor.matmul(warm[:, 128:256], cpack[:, 13:29],
                               bcast_free(cpack[:, 0:1], 128),
                               start=True, stop=True, skip_group_check=True)

              # ---- X: one DMA per col-tile (all four K-chunks) ----
              xt_pack = cp.tile([128, 4, W], bf16, tag="xtp", name="xt_pack")
              xt = [xt_pack[:, k, :] for k in range(4)]
              xt_k = xt_d.rearrange("(k p) w -> p k w", p=128)
              for s, w in CTS:
                  nc.sync.dma_start(out=xt_pack[:, :, s:s + w],
                                    in_=xt_k[:, :, s:s + w])

              # ---- weight views (all bf16) ----
              wfc1 = [wpack[:, 128 * k:128 * (k + 1)] for k in range(4)]
              wz = {"f": wpack[:, 512:640], "r": wpack[:, 640:768]}
              wzb = {"f": wpack[:, 768:832], "r": wpack[:, 832:896]}
              wz1hi = {"f": wpack[0:64, 896:1024], "r": wpack[64:128, 896:1024]}
              wz1lo = {"f": wpack[0:64, 1024:1152], "r": wpack[64:128, 1024:1152]}
              wz1b = {"f": wpack[:, 1152:1216], "r": wpack[:, 1216:1280]}
              w2d = wpack[:, 1280:1296]
              w2s0 = wpack[:, 1296:1312]
              w2s1 = wpack[:, 1312:1328]

              # ---- persistent activation tiles ----
              D = ap_.tile([128, W], bf16, tag="D")
              Z = ap_.tile([128, W], bf16, tag="Z")    # [0:64]=z_f, [64:128]=z_r
              ZZ = ap_.tile([128, W], bf16, tag="ZZ")  # z1_f, z1_r
              P13 = ap_.tile([128, W], f32, tag="P13")
              P25 = ap_.tile([128, W], f32, tag="P25")
              S0 = ap_.tile([128, W], bf16, tag="S0")  # [0:64]=s0f, [64:128]=s0r
              S1 = ap_.tile([128, W], bf16, tag="S1")  # [0:64]=s1f, [64:128]=s1r
              Lsb = ap_.tile([NCLS, W], f32, tag="Lsb")
              LT = ap_.tile([128, NT, NCLS], f32, tag="LT")
              se = ap_.tile([128, NT], f32, tag="se")

              # edge lanes the window subs can't reach (values unused; they
              # only need to be finite so the chained scans stay clean)
              nc.gpsimd.memset(S0[0:64, 0:1], 0.0)
              nc.gpsimd.memset(S0[64:128, 0:13], 0.0)
              nc.gpsimd.memset(S1[0:64, 0:1], 0.0)
              nc.gpsimd.memset(S1[64:128, 0:25], 0.0)

              # ---- stage A+B per col-tile: fc1, layer-0 MLP, chained scan --
              def stage_ab(j):
                  s, w = CTS[j]
                  psA = pmm.tile([128, 2, 512], f32, tag="mm", name="psA")
                  for k in range(4):
                      nc.tensor.matmul(psA[:, 0, :w], wfc1[k], xt[k][:, s:s + w],
                                       start=(k == 0), stop=(k == 3))
                  nc.scalar.activation(D[:, s:s + w], psA[:, 0, :w], AF.Relu,
                                       bias=bfc1)
                  t1p = pmm.tile([128, 2, 512], f32, tag="mm", name="t1p")
                  nc.tensor.matmul(t1p[:, 0, :w], wz["f"], D[:, s:s + w],
                                   start=True, stop=True)
                  nc.tensor.matmul(t1p[:, 1, :w], wz["r"], D[:, s:s + w],
                                   start=True, stop=True)
                  t1s = sp.tile([128, 2, 512], bf16, tag="t1s", name="t1s")
                  nc.scalar.activation(t1s[:, :, :w], t1p[:, :, :w], AF.Relu)
                  zp = pzz.tile([128, 512], f32, tag="zz", name="zp")
                  nc.tensor.matmul(zp[0:64, :w], wzb["f"], t1s[:, 0, :w],
                                   start=True, stop=True)
                  nc.tensor.matmul(zp[64:128, :w], wzb["r"], t1s[:, 1, :w],
                                   start=True, stop=True)
                  nc.vector.tensor_scalar_max(Z[:, s:s + w], zp[:, :w], 0.0)
                  nc.vector.tensor_tensor_scan(
                      P13[:, s:s + w], Z[:, s:s + w], Z[:, s:s + w],
                      initial=(0.0 if s == 0 else P13[:, s - 1:s]),
                      op0=OP.add, op1=OP.bypass)

              # ---- stage C: s0 windows from shifted prefix differences ----
              def win13(j):
                  s, w = CTS[j]
                  a, b = (1 if s == 0 else s), min(s + w, 1332)
                  nc.vector.tensor_sub(S0[0:64, a:b], P13[0:64, a + 12:b + 12],
                                       P13[0:64, a - 1:b - 1])
                  a, b = (13 if s == 0 else s), s + w
                  nc.gpsimd.tensor_sub(S0[64:128, a:b], P13[64:128, a:b],
                                       P13[64:128, a - 13:b - 13])

              # ---- stage D: layer-1 MLP (split contraction) + chained scan -
              def stage_d(j):
                  s, w = CTS[j]
                  t2p = pmm.tile([128, 2, 512], f32, tag="mm", name="t2p")
                  for i, br in enumerate("fr"):
                      h = slice(0, 64) if br == "f" else slice(64, 128)
                      nc.tensor.matmul(t2p[:, i, :w], wz1hi[br], S0[h, s:s + w],
                                       start=True, stop=False)
                      nc.tensor.matmul(t2p[:, i, :w], wz1lo[br], Z[h, s:s + w],
                                       start=False, stop=True)
                  t2s = sp.tile([128, 2, 512], bf16, tag="t1s", name="t2s")
                  nc.scalar.activation(t2s[:, :, :w], t2p[:, :, :w], AF.Relu)
                  z1p = pzz.tile([128, 512], f32, tag="zz", name="z1p")
                  nc.tensor.matmul(z1p[0:64, :w], wz1b["f"], t2s[:, 0, :w],
                                   start=True, stop=True)
                  nc.tensor.matmul(z1p[64:128, :w], wz1b["r"], t2s[:, 1, :w],
                                   start=True, stop=True)
                  nc.vector.tensor_scalar_max(ZZ[:, s:s + w], z1p[:, :w], 0.0)
                  nc.vector.tensor_tensor_scan(
                      P25[:, s:s + w], ZZ[:, s:s + w], ZZ[:, s:s + w],
                      initial=(0.0 if s == 0 else P25[:, s - 1:s]),
                      op0=OP.add, op1=OP.bypass)

              # ---- stage E: s1 windows ----
              def win25(j):
                  s, w = CTS[j]
                  a, b = (1 if s == 0 else s), min(s + w, 1320)
                  nc.gpsimd.tensor_sub(S1[0:64, a:b], P25[0:64, a + 24:b + 24],
                                       P25[0:64, a - 1:b - 1])
                  a, b = (25 if s == 0 else s), s + w
                  nc.gpsimd.tensor_sub(S1[64:128, a:b], P25[64:128, a:b],
                                       P25[64:128, a - 25:b - 25])

              # ---- stage F: fc2 + logits eviction ----
              BLKS = ((0, 4), (4, 4), (8, 3))  # (first 128-block, n blocks)

              def bcast(t2d, n):
                  return bass.AP(tensor=t2d.tensor, offset=t2d.offset,
                                 ap=[t2d.ap[0], [t2d.ap[1][0], n], [0, NCLS]])

              psTs = {}

              def stage_f(j):
                  s, w = CTS[j]
                  psl = pl.tile([NCLS, 512], f32, tag="psL", name="psl")
                  for i, (wc, rhs) in enumerate((
                          (w2d, D[:, s:s + w]),
                          (w2s0, S0[:, s:s + w]),
                          (w2s1, S1[:, s:s + w]))):
                      nc.tensor.matmul(psl[:, :w], wc, rhs,
                                       start=(i == 0), stop=(i == 2))
                  nc.scalar.activation(Lsb[:, s:s + w], psl[:, :w], AF.Identity,
                                       bias=bfc2)

              # ---- stage G: transpose, log_softmax (no max-sub), out DMA ----
              def softmax_chunk(psT, t0, c0, nb):
                  # psT holds blocks [t0, t0+...); this chunk covers
                  # blocks [t0+c0, t0+c0+nb)
                  ex = sp.tile([128, 4, NCLS], f32, tag="ex", name="ex")
                  seh = se[:, t0 + c0:t0 + c0 + nb]
                  nc.scalar.activation(ex[:, 0:nb, :], psT[:, c0:c0 + nb, :],
                                       AF.Exp)
                  nc.vector.reduce_sum(seh, ex[:, 0:nb, :],
                                       axis=mybir.AxisListType.X)
                  nc.scalar.activation(seh, seh, AF.Ln)
                  lt = LT[:, t0 + c0:t0 + c0 + nb, :]
                  nc.vector.tensor_sub(lt, psT[:, c0:c0 + nb, :], bcast(seh, nb))
                  nc.gpsimd.tensor_mul(
                      lt, lt, bcast(maskv[:, t0 + c0:t0 + c0 + nb], nb))
                  nc.sync.dma_start(out=out_d[:, t0 + c0:t0 + c0 + nb, :],
                                    in_=lt)

              def stage_g_transpose(j):
                  t0, nb = BLKS[j]
                  psT = pt.tile([128, 4, NCLS], f32, tag="psT", name="psT")
                  psTs[j] = psT
                  if j == 2:
                      nc.vector.memset(psT[64:128, nb - 1, :], 0.0)
                  for t in range(t0, t0 + nb):
                      wt = 128 if t < NT - 1 else W - 128 * (NT - 1)
                      nc.tensor.transpose(psT[:wt, t - t0, :],
                                          Lsb[:, 128 * t:128 * t + wt], ident)

              def stage_g_softmax(j):
                  t0, nb = BLKS[j]
                  if j < 2:
                      softmax_chunk(psTs[j], t0, 0, nb)
                  else:
                      softmax_chunk(psTs[j], t0, 0, nb - 1)
                      softmax_chunk(psTs[j], t0, nb - 1, 1)

              # ---- emission order: dense PE stream, one-tile-lag windows ----
              stage_ab(0)
              stage_ab(1)
              win13(0)
              stage_ab(2)
              win13(1)
              stage_d(0)
              win13(2)
              stage_d(1)
              win25(0)
              stage_d(2)
              stage_f(0)
              win25(1)
              stage_f(1)
              stage_g_transpose(0)
              stage_g_softmax(0)
              win25(2)
              stage_f(2)
              stage_g_transpose(1)
              stage_g_softmax(1)
              stage_g_transpose(2)
              stage_g_softmax(2)

    # Steer the ACT-table pass to natural_log_exp_and_others (covers Relu,
    # Identity, Copy, Exp AND Ln) so the kernel pays one table load instead
    # of a ~2.7us mid-kernel switch before the final Ln.
    import concourse.bacc as bacc_mod
    AFt = mybir.ActivationFunctionType

    orig_tables = bacc_mod.get_activation_tables
    mine = {AFt.Relu, AFt.Identity, AFt.Copy, AFt.Exp, AFt.Ln}

    def steered(arch):
        t = orig_tables(arch)
        out = {}
        seen_pref = False
        for name, fns in t.items():
            if name == "natural_log_exp_and_others":
                seen_pref = True
                out[name] = fns
            elif not seen_pref:
                out[name] = type(fns)(f for f in fns if f not in mine)
            else:
                out[name] = fns
        return out

    bacc_mod.get_activation_tables = steered
    try:
        nc.compile()
    finally:
        bacc_mod.get_activation_tables = orig_tables
    return nc


def _get_nc(repeat=1):
    global _NC
    if repeat != 1:
        return _build_nc(repeat)
    if _NC is None:
        _NC = _build_nc()
    return _NC


# --------------------------------------------------------------------------
# host-side sharding + entry point
# --------------------------------------------------------------------------

def _make_in_maps(inputs):
    sf = np.asarray(inputs["sparse_feature"], dtype=_F32)
    maskf = np.asarray(inputs["mask"]).astype(_F32)

    def f(k):
        return np.asarray(inputs[k], dtype=_F32)

    mw0a, rw0a = f("mw0a"), f("rw0a")
    wpack = np.zeros((128, WPACK_COLS), dtype=_F32)
    wpack[:, 0:512] = f("fc1_w").reshape(4, 128, C_HID).transpose(1, 0, 2) \
        .reshape(128, 512)
    wpack[:, 512:640] = (mw0a[:C_HID] + mw0a[C_HID:]) / RF1
    wpack[:, 640:768] = (rw0a[:C_HID] + rw0a[C_HID:]) / RF1
    wpack[:, 768:832] = f("mw0b")
    wpack[:, 832:896] = f("rw0b")
    mw1a, rw1a = f("mw1a") / RF2, f("rw1a") / RF2
    wpack[0:64, 896:1024] = mw1a[0:64]      # s0-part, branch f (parts 0:64)
    wpack[64:128, 896:1024] = rw1a[0:64]    # s0-part, branch r (parts 64:128)
    wpack[0:64, 1024:1152] = mw1a[64:128]   # z-part, branch f
    wpack[64:128, 1024:1152] = rw1a[64:128]  # z-part, branch r
    wpack[:, 1152:1216] = f("mw1b")
    wpack[:, 1216:1280] = f("rw1b")
    w2 = f("fc2_w")
    wpack[:, 1280:1296] = w2[0:128]
    wpack[0:64, 1296:1312] = w2[128:192]    # s0f rows
    wpack[64:128, 1296:1312] = w2[256:320]  # s0r rows
    wpack[0:64, 1312:1328] = w2[192:256]    # s1f rows
    wpack[64:128, 1312:1328] = w2[320:384]  # s1r rows
    wpack16 = wpack.astype(_BF16)

    cbase = np.zeros((128, CPACK_COLS), dtype=_F32)
    cbase[:, 0] = f("fc1_b")
    cbase[0:NCLS, 1] = f("fc2_b")
    cbase[0:NCLS, 2 + NT:2 + NT + NCLS] = np.eye(NCLS, dtype=_F32)

    in_maps = []
    for c in range(NCORES):
        b = c * BLK
        idx = (b - HALO + np.arange(W)) % N
        xt = np.ascontiguousarray(sf[idx].T.astype(_BF16))
        me = np.zeros(128 * NT, dtype=_F32)
        me[:W] = maskf[idx]
        cpack = cbase.copy()
        cpack[:, 2:2 + NT] = me.reshape(NT, 128).T
        in_maps.append({"wpack": wpack16, "cpack": cpack, "xt": xt})
    return in_maps


_RUNNER = None


def _make_runner():
    """Build the 8-core PJRT executor once; reuse across kernel() calls."""
    import jax
    from jax.sharding import Mesh, NamedSharding, PartitionSpec
    from jax.experimental.shard_map import shard_map
    from concourse import mybir
    from concourse.bass2jax import (_bass_exec_p, install_neuronx_cc_hook,
                                    partition_id_tensor)

    nc = _get_nc()
    install_neuronx_cc_hook()
    in_names, out_names, out_avals, zero_shapes = [], [], [], []
    pname = nc.partition_id_tensor.name if nc.partition_id_tensor else None
    for alloc in nc.m.functions[0].allocations:
        if not isinstance(alloc, mybir.MemoryLocationSet):
            continue
        name = alloc.memorylocations[0].name
        if alloc.kind == "ExternalInput":
            if name != pname:
                in_names.append(name)
        elif alloc.kind == "ExternalOutput":
            out_names.append(name)
            shape = tuple(alloc.tensor_shape)
            dtype = mybir.dt.np(alloc.dtype)
            out_avals.append(jax.core.ShapedArray(shape, dtype))
            zero_shapes.append((shape, dtype))
    n_params = len(in_names)
    all_in = list(in_names) + list(out_names)
    if pname is not None:
        all_in.append(pname)
    donate = tuple(range(n_params, n_params + len(out_names)))

    def _body(*args):
        operands = list(args)
        if pname is not None:
            operands.append(partition_id_tensor())
        return tuple(_bass_exec_p.bind(
            *operands,
            out_avals=tuple(out_avals),
            in_names=tuple(all_in),
            out_names=tuple(out_names),
            lowering_input_output_aliases=(),
            sim_require_finite=False,
            sim_require_nnan=False,
            nc=nc,
        ))

    devices = jax.devices()[:NCORES]
    mesh = Mesh(np.asarray(devices), ("core",))
    shd = NamedSharding(mesh, PartitionSpec("core"))
    n_outs = len(out_names)
    sharded = jax.jit(
        shard_map(_body, mesh=mesh,
                  in_specs=(PartitionSpec("core"),) * (n_params + n_outs),
                  out_specs=(PartitionSpec("core"),) * n_outs,
                  check_rep=False),
        donate_argnums=donate, keep_unused=True,
    )

    def run(in_maps):
        concat_in = [
            np.concatenate([np.asarray(in_maps[c][nm]) for c in range(NCORES)],
                           axis=0)
            for nm in in_names
        ]
        dev_in = [jax.device_put(x, shd) for x in concat_in]
        zeros = [
            jax.device_put(np.zeros((NCORES * s[0], *s[1:]), dt), shd)
            for s, dt in zero_shapes
        ]
        outs = sharded(*dev_in, *zeros)
        res = np.asarray(outs[out_names.index("out")])
        # res: [NCORES*128, NT, 16] -> per core [128, NT, 16]; lane = t*128+p
        res = res.reshape(NCORES, 128, NT, NCLS).transpose(0, 2, 1, 3) \
                 .reshape(NCORES, NT * 128, NCLS)
        return np.ascontiguousarray(res[:, LO:HI, :]).reshape(N, NCLS)

    return run


def kernel(**inputs):
    if not _structure_matches(inputs):
        return _reference_numpy(inputs)
    global _RUNNER
    if _RUNNER is None:
        _RUNNER = _make_runner()
    return _RUNNER(_make_in_maps(inputs))


# revision 27
# speedup vs baseline: 8.7877x; 2.2676x over previous
"""Trainium2 Bass kernel for nn_CCN1D (circulant GNN message passing).

Strategy
--------
The reference gathers receptive fields on a circulant ring graph and runs
per-edge MLPs followed by segment sums.  Because every gathered row's MLP
output depends only on the *source* vertex, the per-edge MLPs (130k / 250k
rows) collapse to per-vertex MLPs (10k rows) plus sliding-window sums along
the ring:

    dense = relu(X @ W1 + b1)                           [N, 128]
    z_f[u]  = relu(relu(dense[u] @ (w0a_lo+w0a_hi)/13) @ w0b)      [N, 64]
    s0_f[v] = sum_{j=0..12} z_f[(v+j) % N]              (window sum)
    z1_f[u] = relu(relu(concat(s0_f[u], z_f[u])/25 @ w1a) @ w1b)
    s1_f[v] = sum_{j=0..24} z1_f[(v+j) % N]
    (reverse branch identical with backward windows)
    logits  = concat(dense, s0f, s1f, s0r, s1r) @ W2 + b2
    out     = log_softmax(logits) * mask

Sharding: vertices are range-partitioned across 8 cores with a 36-vertex
halo on each side (graph/data parallel; weights replicated; no device
collectives needed - the halo makes every core self-sufficient).

v2 implementation notes (vs the earlier f32r version):
- all matmul operands are bf16 (X, weights, activations): halves all HBM
  traffic; window prefix sums stay f32.  rel-err ~1.5e-3, well inside 2e-2.
- layer-1 contraction split into two 64-row matmuls (s0-part from the S0
  tile, z-part from the Z tile) so the s0f/s0r window outputs live in one
  128-partition tile; fc2 then needs only 3 passes (D, S0, S1).
- branch f/r PSUM outputs paired in one two-bank PSUM tile so a single
  activation op evicts both.
- log_softmax skips the max-subtraction (fp32 exp is safe at this logit
  scale) and runs per col-tile on the transposed PSUM banks.
- output written per-partition-contiguous ([128, 11, 16]); the host
  de-interleaves lanes back to row-major.
- edge lanes handled by tiny memsets; out-of-range lanes are discarded by
  the host gather.
"""

import sys

import numpy as np

for _p in ("/opt/trn_rl_repo",):
    if _p not in sys.path:
        sys.path.insert(0, _p)

import ml_dtypes

N = 10000
NCORES = 8
BLK = N // NCORES          # 1250 vertices per core
HALO = 36                  # 12 (layer-0 window) + 24 (layer-1 window)
W = 1322                   # on-chip free width (exactly the lanes needed)
NT = 11                    # 128-lane row tiles covering W (10*128 + 64)
CTS = ((0, 512), (512, 512), (1024, 298))
RF1, RF2 = 13, 25
C_IN, C_HID, MLP_H, MSG, NCLS = 512, 128, 128, 64, 16
LO, HI = HALO, HALO + BLK  # valid output lane range [36, 1286)
WPACK_COLS = 1328          # packed bf16 matmul weights
CPACK_COLS = 2 + NT + NCLS  # biases + mask + identity (f32)

_F32 = np.float32
_BF16 = ml_dtypes.bfloat16


# --------------------------------------------------------------------------
# structure check (is the input the circulant graph the kernel was built for?)
# --------------------------------------------------------------------------

def _expected_idx():
    v = np.arange(N)
    return {
        "f_rf1": ((v[:, None] + np.arange(RF1)) % N).reshape(-1),
        "f_rf2": ((v[:, None] + np.arange(RF2)) % N).reshape(-1),
        "r_rf1": ((v[:, None] - np.arange(RF1)) % N).reshape(-1),
        "r_rf2": ((v[:, None] - np.arange(RF2)) % N).reshape(-1),
        "own1": np.repeat(v, RF1),
        "own2": np.repeat(v, RF2),
        "self1": v * RF1,
    }


def _structure_matches(inputs):
    try:
        if inputs["sparse_feature"].shape != (N, C_IN):
            return False
        for k, exp in _expected_idx().items():
            got = np.asarray(inputs[k])
            if got.shape != exp.shape or not np.array_equal(got, exp):
                return False
        return True
    except Exception:
        return False


# --------------------------------------------------------------------------
# generic numpy fallback (exact reference semantics, any index content)
# --------------------------------------------------------------------------

def _segment_sum(data, seg, num):
    out = np.zeros((num,) + data.shape[1:], dtype=data.dtype)
    np.add.at(out, seg, data)
    return out


def _np_branch(dense, rf1, rf2, own1, own2, self1, w0a, w0b, w1a, w1b):
    sizes1 = _segment_sum(np.ones(own1.shape, dense.dtype), own1, N)
    sizes2 = _segment_sum(np.ones(own2.shape, dense.dtype), own2, N)
    g = dense[rf1]
    m0 = np.concatenate([g, g], axis=-1) / sizes1[own1][:, None]
    h0 = np.maximum(np.maximum(m0 @ w0a, 0.0) @ w0b, 0.0)
    s0 = _segment_sum(h0, own1, N)
    selfr = h0[self1]
    m1 = np.concatenate([s0[rf2], selfr[rf2]], axis=-1) / sizes2[own2][:, None]
    h1 = np.maximum(np.maximum(m1 @ w1a, 0.0) @ w1b, 0.0)
    s1 = _segment_sum(h1, own2, N)
    return s0, s1


def _reference_numpy(inputs):
    f = {k: np.asarray(v) for k, v in inputs.items()}
    dense = np.maximum(
        f["sparse_feature"].astype(_F32) @ f["fc1_w"] + f["fc1_b"], 0.0
    )
    s0f, s1f = _np_branch(dense, f["f_rf1"], f["f_rf2"], f["own1"], f["own2"],
                          f["self1"], f["mw0a"], f["mw0b"], f["mw1a"], f["mw1b"])
    s0r, s1r = _np_branch(dense, f["r_rf1"], f["r_rf2"], f["own1"], f["own2"],
                          f["self1"], f["rw0a"], f["rw0b"], f["rw1a"], f["rw1b"])
    total = np.concatenate([dense, s0f, s1f, s0r, s1r], axis=1)
    logits = total @ f["fc2_w"] + f["fc2_b"]
    m = logits.max(axis=-1, keepdims=True)
    lse = m + np.log(np.exp(logits - m).sum(axis=-1, keepdims=True))
    return ((logits - lse) * f["mask"][:, None].astype(_F32)).astype(_F32)


# --------------------------------------------------------------------------
# device kernel
# --------------------------------------------------------------------------

_NC = None


def _build_nc(repeat=1):
    import concourse.bass as bass
    import concourse.tile as tile
    from concourse import bacc, mybir

    f32 = mybir.dt.float32
    bf16 = mybir.dt.bfloat16
    AF = mybir.ActivationFunctionType
    OP = mybir.AluOpType

    nc = bacc.Bacc(trn_type="TRN2", debug=False)

    xt_d = nc.dram_tensor("xt", [C_IN, W], bf16, kind="ExternalInput").ap()
    wpack_d = nc.dram_tensor("wpack", [128, WPACK_COLS], bf16,
                             kind="ExternalInput").ap()
    cpack_d = nc.dram_tensor("cpack", [128, CPACK_COLS], f32,
                             kind="ExternalInput").ap()
    out_d = nc.dram_tensor("out", [128, NT, NCLS], f32,
                           kind="ExternalOutput").ap()

    with tile.TileContext(nc) as tc:
        from contextlib import ExitStack

        with ExitStack() as ctx:
            cp = ctx.enter_context(tc.tile_pool(name="consts", bufs=1))
            ap_ = ctx.enter_context(tc.tile_pool(name="acts", bufs=1))
            sp = ctx.enter_context(tc.tile_pool(name="scr", bufs=3))
            pmm = ctx.enter_context(tc.tile_pool(name="pmm", bufs=2, space="PSUM"))
            pzz = ctx.enter_context(tc.tile_pool(name="pzz", bufs=1, space="PSUM"))
            pl = ctx.enter_context(tc.tile_pool(name="pl", bufs=1, space="PSUM"))
            pt = ctx.enter_context(tc.tile_pool(name="pt", bufs=2, space="PSUM"))

            for _rep in range(repeat):
              # ---- const DMAs first (tiny cpack feeds the PE warm-up) ----
              cpack = cp.tile([128, CPACK_COLS], f32, tag="cpack", name="cpack")
              nc.sync.dma_start(out=cpack, in_=cpack_d)
              wpack = cp.tile([128, WPACK_COLS], bf16, tag="wpack", name="wpack")
              nc.sync.dma_start(out=wpack, in_=wpack_d)

              bfc1 = cpack[:, 0:1]
              bfc2h = [cpack[0:16, 1:2], cpack[32:48, 1:2]]
              maskv = cpack[:, 2:2 + NT]
              ident = cpack[0:16, 2 + NT:2 + NT + NCLS]

              # PE warm-up on cpack garbage (f32 = 4 cycles/row keeps the PE
              # busy through its p-state ramp while X streams in).  ifmap is
              # a stride-0 broadcast of one cpack column.
              def bcast_free(t2d, m):
                  return bass.AP(tensor=t2d.tensor, offset=t2d.offset,
                                 ap=[t2d.ap[0], [0, m]])

              # a couple of dummy passes nudge the PE p-state ramp while the
              # consts stream in; kept short so a faster-than-modeled X DMA
              # is never stuck behind them (the PE queue is in-order)
              warm = pl.tile([NCLS, 512], f32, tag="psL", name="warm")
              for wi in range(2):
                  c = 128 * (wi % 2)
                  nc.tensor.matmul(warm[:, c:c + 128], cpack[:, 13:29],
                                   bcast_free(cpack[:, 0:1], 128),
                                   start=True, stop=True,
                                   skip_group_check=True)

              # ---- X: one DMA per col-tile (all four K-chunks) ----
              xt_pack = cp.tile([128, 4, W], bf16, tag="xtp", name="xt_pack")
              xt = [xt_pack[:, k, :] for k in range(4)]
              xt_k = xt_d.rearrange("(k p) w -> p k w", p=128)
              for s, w in CTS:
                  nc.sync.dma_start(out=xt_pack[:, :, s:s + w],
                                    in_=xt_k[:, :, s:s + w])

              # ---- weight views (all bf16) ----
              wfc1 = [wpack[:, 128 * k:128 * (k + 1)] for k in range(4)]
              wz = {"f": wpack[:, 512:640], "r": wpack[:, 640:768]}
              wzb = {"f": wpack[:, 768:832], "r": wpack[:, 832:896]}
              wz1hi = {"f": wpack[0:64, 896:1024], "r": wpack[64:128, 896:1024]}
              wz1lo = {"f": wpack[0:64, 1024:1152], "r": wpack[64:128, 1024:1152]}
              wz1b = {"f": wpack[:, 1152:1216], "r": wpack[:, 1216:1280]}
              w2d = wpack[:, 1280:1296]
              w2s0 = wpack[:, 1296:1312]
              w2s1 = wpack[:, 1312:1328]

              # ---- persistent activation tiles ----
              D = ap_.tile([128, W], bf16, tag="D")
              Z = ap_.tile([128, W], bf16, tag="Z")    # [0:64]=z_f, [64:128]=z_r
              ZZ = ap_.tile([128, W], bf16, tag="ZZ")  # z1_f, z1_r
              P13 = ap_.tile([128, W], f32, tag="P13")
              P25 = ap_.tile([128, W], f32, tag="P25")
              S0 = ap_.tile([128, W], bf16, tag="S0")  # [0:64]=s0f, [64:128]=s0r
              S1 = ap_.tile([128, W], bf16, tag="S1")  # [0:64]=s1f, [64:128]=s1r
              Lsb = ap_.tile([NCLS, W], f32, tag="Lsb")
              LT = ap_.tile([128, NT, NCLS], f32, tag="LT")
              se = ap_.tile([128, NT], f32, tag="se")

              # edge lanes the window subs can't reach (values unused; they
              # only need to be finite so the chained scans stay clean)
              nc.gpsimd.memset(S0[0:64, 0:1], 0.0)
              nc.gpsimd.memset(S0[64:128, 0:13], 0.0)
              nc.gpsimd.memset(S1[0:64, 0:1], 0.0)
              nc.gpsimd.memset(S1[64:128, 0:25], 0.0)

              # ---- stage A+B per col-tile: fc1, layer-0 MLP, chained scan --
              def stage_ab(j):
                  s, w = CTS[j]
                  psA = pmm.tile([128, 2, 512], f32, tag="mm", name="psA")
                  for k in range(4):
                      nc.tensor.matmul(psA[:, 0, :w], wfc1[k], xt[k][:, s:s + w],
                                       start=(k == 0), stop=(k == 3))
                  if j == 2:
                      # tile 2 rebalanced to DVE: relu(x + b) as fused
                      # tensor_scalar (per-partition bias)
                      nc.vector.tensor_scalar(D[:, s:s + w], psA[:, 0, :w],
                                              bfc1, 0.0, op0=OP.add,
                                              op1=OP.max)
                  else:
                      nc.scalar.activation(D[:, s:s + w], psA[:, 0, :w],
                                           AF.Relu, bias=bfc1)
                  t1p = pmm.tile([128, 2, 512], f32, tag="mm", name="t1p")
                  nc.tensor.matmul(t1p[:, 0, :w], wz["f"], D[:, s:s + w],
                                   start=True, stop=True)
                  nc.tensor.matmul(t1p[:, 1, :w], wz["r"], D[:, s:s + w],
                                   start=True, stop=True)
                  t1s = sp.tile([128, 2, 512], bf16, tag="t1s", name="t1s")
                  nc.scalar.activation(t1s[:, :, :w], t1p[:, :, :w], AF.Relu)
                  zp = pzz.tile([128, 512], f32, tag="zz", name="zp")
                  nc.tensor.matmul(zp[0:64, :w], wzb["f"], t1s[:, 0, :w],
                                   start=True, stop=True)
                  nc.tensor.matmul(zp[64:128, :w], wzb["r"], t1s[:, 1, :w],
                                   start=True, stop=True)
                  nc.vector.tensor_scalar_max(Z[:, s:s + w], zp[:, :w], 0.0)
                  nc.vector.tensor_tensor_scan(
                      P13[:, s:s + w], Z[:, s:s + w], Z[:, s:s + w],
                      initial=(0.0 if s == 0 else P13[:, s - 1:s]),
                      op0=OP.add, op1=OP.bypass)

              # ---- stage C: s0 windows from shifted prefix differences ----
              def win13(j):
                  s, w = CTS[j]
                  a, b = (1 if s == 0 else s), min(s + w, W - 12)
                  eng = nc.gpsimd if j == 1 else nc.vector
                  eng.tensor_sub(S0[0:64, a:b], P13[0:64, a + 12:b + 12],
                                 P13[0:64, a - 1:b - 1])
                  a, b = (13 if s == 0 else s), s + w
                  nc.gpsimd.tensor_sub(S0[64:128, a:b], P13[64:128, a:b],
                                       P13[64:128, a - 13:b - 13])

              # ---- stage D: layer-1 MLP (split contraction) + chained scan -
              def stage_d(j):
                  s, w = CTS[j]
                  t2p = pmm.tile([128, 2, 512], f32, tag="mm", name="t2p")
                  for i, br in enumerate("fr"):
                      h = slice(0, 64) if br == "f" else slice(64, 128)
                      nc.tensor.matmul(t2p[:, i, :w], wz1hi[br], S0[h, s:s + w],
                                       start=True, stop=False)
                      nc.tensor.matmul(t2p[:, i, :w], wz1lo[br], Z[h, s:s + w],
                                       start=False, stop=True)
                  t2s = sp.tile([128, 2, 512], bf16, tag="t1s", name="t2s")
                  nc.scalar.activation(t2s[:, :, :w], t2p[:, :, :w], AF.Relu)
                  z1p = pzz.tile([128, 512], f32, tag="zz", name="z1p")
                  nc.tensor.matmul(z1p[0:64, :w], wz1b["f"], t2s[:, 0, :w],
                                   start=True, stop=True)
                  nc.tensor.matmul(z1p[64:128, :w], wz1b["r"], t2s[:, 1, :w],
                                   start=True, stop=True)
                  nc.vector.tensor_scalar_max(ZZ[:, s:s + w], z1p[:, :w], 0.0)
                  nc.vector.tensor_tensor_scan(
                      P25[:, s:s + w], ZZ[:, s:s + w], ZZ[:, s:s + w],
                      initial=(0.0 if s == 0 else P25[:, s - 1:s]),
                      op0=OP.add, op1=OP.bypass)

              # ---- stage E: s1 windows ----
              def win25(j):
                  s, w = CTS[j]
                  a, b = (1 if s == 0 else s), min(s + w, W - 24)
                  # tile 2 is tail-critical: run its f-half on DVE so both
                  # halves land in parallel
                  eng = nc.vector if j == 2 else nc.gpsimd
                  eng.tensor_sub(S1[0:64, a:b], P25[0:64, a + 24:b + 24],
                                 P25[0:64, a - 1:b - 1])
                  a, b = (25 if s == 0 else s), s + w
                  nc.gpsimd.tensor_sub(S1[64:128, a:b], P25[64:128, a:b],
                                       P25[64:128, a - 25:b - 25])

              # ---- stage F: fc2 + logits eviction ----
              BLKS = ((0, 4), (4, 4), (8, 3))  # (first 128-block, n blocks)

              def bcast(t2d, n):
                  return bass.AP(tensor=t2d.tensor, offset=t2d.offset,
                                 ap=[t2d.ap[0], [t2d.ap[1][0], n], [0, NCLS]])

              psTs = {}

              psl_bank = {}

              def stage_f(j):
                  s, w = CTS[j]
                  # manual double-buffer: consecutive tiles use partition
                  # halves [0:16] / [16:32] of one PSUM bank, so fc2(j+1)
                  # never waits for Lsb(j)'s eviction
                  if "t" not in psl_bank:
                      psl_bank["t"] = pl.tile([48, 512], f32, tag="psL",
                                              name="pslb")
                  h = 32 * (j % 2)
                  psl = psl_bank["t"][h:h + NCLS, :]
                  for i, (wc, rhs) in enumerate((
                          (w2d, D[:, s:s + w]),
                          (w2s0, S0[:, s:s + w]),
                          (w2s1, S1[:, s:s + w]))):
                      nc.tensor.matmul(psl[:, :w], wc, rhs,
                                       start=(i == 0), stop=(i == 2))
                  bfc2 = bfc2h[j % 2]
                  if j == 1:
                      # rebalanced to DVE: logits + per-partition bias
                      nc.vector.tensor_scalar_add(Lsb[:, s:s + w], psl[:, :w],
                                                  bfc2)
                  else:
                      nc.scalar.activation(Lsb[:, s:s + w], psl[:, :w],
                                           AF.Identity, bias=bfc2)

              # ---- stage G: transpose, log_softmax (no max-sub), out DMA ----
              def softmax_chunk(psT, t0, c0, nb, last=False):
                  # psT holds blocks [t0, t0+...); this chunk covers
                  # blocks [t0+c0, t0+c0+nb)
                  ex = sp.tile([128, 4, NCLS], f32, tag="ex", name="ex")
                  seh = se[:, t0 + c0:t0 + c0 + nb]
                  nc.scalar.activation(ex[:, 0:nb, :], psT[:, c0:c0 + nb, :],
                                       AF.Exp)
                  nc.vector.reduce_sum(seh, ex[:, 0:nb, :],
                                       axis=mybir.AxisListType.X)
                  nc.scalar.activation(seh, seh, AF.Ln)
                  lt = LT[:, t0 + c0:t0 + c0 + nb, :]
                  nc.vector.tensor_sub(lt, psT[:, c0:c0 + nb, :], bcast(seh, nb))
                  # last tile: keep the mask-mul on DVE (same queue as the
                  # sub, no cross-engine hop on the kernel tail)
                  eng = nc.vector if last else nc.gpsimd
                  eng.tensor_mul(
                      lt, lt, bcast(maskv[:, t0 + c0:t0 + c0 + nb], nb))
                  nc.sync.dma_start(out=out_d[:, t0 + c0:t0 + c0 + nb, :],
                                    in_=lt)

              def stage_g_transpose(j):
                  t0, nb = BLKS[j]
                  psT = pt.tile([128, 4, NCLS], f32, tag="psT", name="psT")
                  psTs[j] = psT
                  if j == 2:
                      nc.vector.memset(psT[64:128, nb - 1, :], 0.0)
                  for t in range(t0, t0 + nb):
                      wt = 128 if t < NT - 1 else W - 128 * (NT - 1)
                      nc.tensor.transpose(psT[:wt, t - t0, :],
                                          Lsb[:, 128 * t:128 * t + wt], ident)

              def stage_g_softmax(j):
                  t0, nb = BLKS[j]
                  softmax_chunk(psTs[j], t0, 0, nb, last=(j == 2))

              # ---- emission order: dense PE stream, one-tile-lag windows ----
              stage_ab(0)
              stage_ab(1)
              win13(0)
              stage_ab(2)
              win13(1)
              stage_d(0)
              win13(2)
              stage_d(1)
              win25(0)
              stage_d(2)
              stage_f(0)
              win25(1)
              stage_f(1)
              stage_g_transpose(0)
              stage_g_softmax(0)
              win25(2)
              stage_f(2)
              stage_g_transpose(1)
              stage_g_softmax(1)
              stage_g_transpose(2)
              stage_g_softmax(2)

    # Steer the ACT-table pass to natural_log_exp_and_others (covers Relu,
    # Identity, Copy, Exp AND Ln) so the kernel pays one table load instead
    # of a ~2.7us mid-kernel switch before the final Ln.
    import concourse.bacc as bacc_mod
    AFt = mybir.ActivationFunctionType

    orig_tables = bacc_mod.get_activation_tables
    mine = {AFt.Relu, AFt.Identity, AFt.Copy, AFt.Exp, AFt.Ln}

    def steered(arch):
        t = orig_tables(arch)
        out = {}
        seen_pref = False
        for name, fns in t.items():
            if name == "natural_log_exp_and_others":
                seen_pref = True
                out[name] = fns
            elif not seen_pref:
                out[name] = type(fns)(f for f in fns if f not in mine)
            else:
                out[name] = fns
        return out

    bacc_mod.get_activation_tables = steered
    try:
        nc.compile()
    finally:
        bacc_mod.get_activation_tables = orig_tables
    return nc


def _get_nc(repeat=1):
    global _NC
    if repeat != 1:
        return _build_nc(repeat)
    if _NC is None:
        _NC = _build_nc()
    return _NC


# --------------------------------------------------------------------------
# host-side sharding + entry point
# --------------------------------------------------------------------------

def _make_in_maps(inputs):
    sf = np.asarray(inputs["sparse_feature"], dtype=_F32)
    maskf = np.asarray(inputs["mask"]).astype(_F32)

    def f(k):
        return np.asarray(inputs[k], dtype=_F32)

    mw0a, rw0a = f("mw0a"), f("rw0a")
    wpack = np.zeros((128, WPACK_COLS), dtype=_F32)
    wpack[:, 0:512] = f("fc1_w").reshape(4, 128, C_HID).transpose(1, 0, 2) \
        .reshape(128, 512)
    wpack[:, 512:640] = (mw0a[:C_HID] + mw0a[C_HID:]) / RF1
    wpack[:, 640:768] = (rw0a[:C_HID] + rw0a[C_HID:]) / RF1
    wpack[:, 768:832] = f("mw0b")
    wpack[:, 832:896] = f("rw0b")
    mw1a, rw1a = f("mw1a") / RF2, f("rw1a") / RF2
    wpack[0:64, 896:1024] = mw1a[0:64]      # s0-part, branch f (parts 0:64)
    wpack[64:128, 896:1024] = rw1a[0:64]    # s0-part, branch r (parts 64:128)
    wpack[0:64, 1024:1152] = mw1a[64:128]   # z-part, branch f
    wpack[64:128, 1024:1152] = rw1a[64:128]  # z-part, branch r
    wpack[:, 1152:1216] = f("mw1b")
    wpack[:, 1216:1280] = f("rw1b")
    w2 = f("fc2_w")
    wpack[:, 1280:1296] = w2[0:128]
    wpack[0:64, 1296:1312] = w2[128:192]    # s0f rows
    wpack[64:128, 1296:1312] = w2[256:320]  # s0r rows
    wpack[0:64, 1312:1328] = w2[192:256]    # s1f rows
    wpack[64:128, 1312:1328] = w2[320:384]  # s1r rows
    wpack16 = wpack.astype(_BF16)

    cbase = np.zeros((128, CPACK_COLS), dtype=_F32)
    cbase[:, 0] = f("fc1_b")
    cbase[0:NCLS, 1] = f("fc2_b")
    cbase[32:32 + NCLS, 1] = f("fc2_b")  # second psl partition-half
    cbase[0:NCLS, 2 + NT:2 + NT + NCLS] = np.eye(NCLS, dtype=_F32)

    in_maps = []
    for c in range(NCORES):
        b = c * BLK
        idx = (b - HALO + np.arange(W)) % N
        xt = np.ascontiguousarray(sf[idx].T.astype(_BF16))
        me = np.zeros(128 * NT, dtype=_F32)
        me[:W] = maskf[idx]
        cpack = cbase.copy()
        cpack[:, 2:2 + NT] = me.reshape(NT, 128).T
        in_maps.append({"wpack": wpack16, "cpack": cpack, "xt": xt})
    return in_maps


_RUNNER = None


def _make_runner():
    """Build the 8-core PJRT executor once; reuse across kernel() calls."""
    import jax
    from jax.sharding import Mesh, NamedSharding, PartitionSpec
    from jax.experimental.shard_map import shard_map
    from concourse import mybir
    from concourse.bass2jax import (_bass_exec_p, install_neuronx_cc_hook,
                                    partition_id_tensor)

    nc = _get_nc()
    install_neuronx_cc_hook()
    in_names, out_names, out_avals, zero_shapes = [], [], [], []
    pname = nc.partition_id_tensor.name if nc.partition_id_tensor else None
    for alloc in nc.m.functions[0].allocations:
        if not isinstance(alloc, mybir.MemoryLocationSet):
            continue
        name = alloc.memorylocations[0].name
        if alloc.kind == "ExternalInput":
            if name != pname:
                in_names.append(name)
        elif alloc.kind == "ExternalOutput":
            out_names.append(name)
            shape = tuple(alloc.tensor_shape)
            dtype = mybir.dt.np(alloc.dtype)
            out_avals.append(jax.core.ShapedArray(shape, dtype))
            zero_shapes.append((shape, dtype))
    n_params = len(in_names)
    all_in = list(in_names) + list(out_names)
    if pname is not None:
        all_in.append(pname)
    donate = tuple(range(n_params, n_params + len(out_names)))

    def _body(*args):
        operands = list(args)
        if pname is not None:
            operands.append(partition_id_tensor())
        return tuple(_bass_exec_p.bind(
            *operands,
            out_avals=tuple(out_avals),
            in_names=tuple(all_in),
            out_names=tuple(out_names),
            lowering_input_output_aliases=(),
            sim_require_finite=False,
            sim_require_nnan=False,
            nc=nc,
        ))

    devices = jax.devices()[:NCORES]
    mesh = Mesh(np.asarray(devices), ("core",))
    shd = NamedSharding(mesh, PartitionSpec("core"))
    n_outs = len(out_names)
    sharded = jax.jit(
        shard_map(_body, mesh=mesh,
                  in_specs=(PartitionSpec("core"),) * (n_params + n_outs),
                  out_specs=(PartitionSpec("core"),) * n_outs,
                  check_rep=False),
        donate_argnums=donate, keep_unused=True,
    )

    def run(in_maps):
        concat_in = [
            np.concatenate([np.asarray(in_maps[c][nm]) for c in range(NCORES)],
                           axis=0)
            for nm in in_names
        ]
        dev_in = [jax.device_put(x, shd) for x in concat_in]
        zeros = [
            jax.device_put(np.zeros((NCORES * s[0], *s[1:]), dt), shd)
            for s, dt in zero_shapes
        ]
        outs = sharded(*dev_in, *zeros)
        res = np.asarray(outs[out_names.index("out")])
        # res: [NCORES*128, NT, 16] -> per core [128, NT, 16]; lane = t*128+p
        res = res.reshape(NCORES, 128, NT, NCLS).transpose(0, 2, 1, 3) \
                 .reshape(NCORES, NT * 128, NCLS)
        return np.ascontiguousarray(res[:, LO:HI, :]).reshape(N, NCLS)

    return run


def kernel(**inputs):
    if not _structure_matches(inputs):
        return _reference_numpy(inputs)
    global _RUNNER
    if _RUNNER is None:
        _RUNNER = _make_runner()
    return _RUNNER(_make_in_maps(inputs))
